# revision 1
# baseline (speedup 1.0000x reference)
"""Trainium2 Bass kernel for a single causal-attention transformer block.

Reference computation (per batch element b):
    xn  = rms_norm(x[b]) * rms_w
    q/k/v = xn @ Wq/Wk/Wv            (16 heads x 128 head dim)
    att = causal_softmax(q k^T / sqrt(2048)) @ v
    out[b] = att @ Wo + x[b]

Sharding (8 NeuronCores): tensor-parallel over heads x data-parallel over
batch.  Core c handles batch b = c // 4 and head-group i = c % 4 (4 heads,
512 columns of Wq/Wk/Wv, 512 rows of Wo).  Each core computes a partial
output  att_i @ Wo_i  for its batch element; the host sums the 4 partials
per batch and adds the residual.

On-device dataflow (per core):
  A. load x^T (bf16), col-sums of squares via ones-matmul -> rstd,
     broadcast rstd via ones-matmul, scale -> xn^T (bf16, resident).
  B. projections: qT/kT [dh, S] per head and v [S, dh] (bf16), fp32 PSUM.
  C. per (head, 512-query-chunk): scoresT tiles [t,s] = kT_tile^T @ qT_chunk,
     exp on ACT (no max-subtract needed: |scores| < ~1), causal mask via
     affine_select on diagonal tiles (with column truncation of the
     fully-masked region), pv-matmul accumulates attn^T [dh, s] in PSUM,
     an M=128 all-ones matmul accumulates the softmax denominator
     pre-broadcast across partitions, then a full-lane approx-reciprocal
     and one multiply normalize during evacuation.
  D. o_proj interleaved per query-chunk: once all heads finish chunk sc,
     out[s,:] += attn_i^T slices @ Wo_i for that chunk's s-tiles, with
     chunked fp32 output DMAs -- so the final matmuls and stores overlap
     the remaining attention work.

All matmul inputs are bf16 (fp32 PSUM accumulation).  rstd from the
RMS-norm is folded into the PSUM evacuation of q/k/v rather than scaling
x^T, and a single shared 8-bank PSUM pool lets the scheduler interleave
the RMS statistics with early projection matmuls.  Measured end-to-end
error vs the fp32 reference is ~9e-4 of the output absmax; measured HW
time ~376 us across 8 cores (slowest core).
"""

import numpy as np
import ml_dtypes

S = 2048          # sequence length
HID = 2048        # hidden dim
NH = 16           # total heads
DH = 128          # head dim
TP = 4            # head-group shards
DP = 2            # batch shards
KSH = HID // TP   # per-core key-dim shard (512)
NHS = KSH // DH   # heads per core (4)
NT = S // 128     # 128-row tiles along s/t/h (16)
NSC = S // 512    # 512-wide chunks along s (4)
EPS = 1e-5
TRUNC = True  # causal truncation of diagonal tiles

BF16 = None  # set lazily (concourse import)
_STATE = {}


def _build_nc():
    from contextlib import ExitStack

    import concourse.bacc as bacc
    import concourse.tile as tile
    from concourse import mybir

    F32 = mybir.dt.float32
    BF = mybir.dt.bfloat16
    AF = mybir.ActivationFunctionType

    nc = bacc.Bacc("TRN2")
    xt = nc.dram_tensor("xt", [HID, S], BF, kind="ExternalInput")
    wq = nc.dram_tensor("wq", [HID, KSH], BF, kind="ExternalInput")
    wk = nc.dram_tensor("wk", [HID, KSH], BF, kind="ExternalInput")
    wv = nc.dram_tensor("wv", [HID, KSH], BF, kind="ExternalInput")
    wo = nc.dram_tensor("wo", [KSH, HID], BF, kind="ExternalInput")
    out = nc.dram_tensor("out", [S, HID], F32, kind="ExternalOutput")

    with tile.TileContext(nc) as tc, ExitStack() as ctx:
        misc = ctx.enter_context(tc.tile_pool(name="misc", bufs=1))
        qt_pool = ctx.enter_context(tc.tile_pool(name="qt", bufs=NHS))
        kt_pool = ctx.enter_context(tc.tile_pool(name="kt", bufs=NHS))
        v_pool = ctx.enter_context(tc.tile_pool(name="v", bufs=NT))
        attn_pool = ctx.enter_context(tc.tile_pool(name="attn", bufs=NHS))
        probs_pool = ctx.enter_context(tc.tile_pool(name="probs", bufs=12))
        denb_pool = ctx.enter_context(tc.tile_pool(name="denb", bufs=4))

        ones_mat_bf = misc.tile([128, 128], BF, tag="ones_mat_bf", name="ones_mat_bf")
        nc.vector.memset(ones_mat_bf, 1.0)
        eps_sb = misc.tile([128, 1], F32, tag="eps_sb", name="eps_sb")
        nc.vector.memset(eps_sb, EPS)
        rstd_b = misc.tile([128, S], F32, tag="rstd_b", name="rstd_b")
        # rstd transposed to per-partition layout: rstd_colT[p, st] = rstd[st*128+p]
        rstd_colT = misc.tile([128, NT], F32, tag="rstd_colT", name="rstd_colT")
        ident = misc.tile([128, 128], F32, tag="ident", name="ident")
        nc.vector.memset(ident, 1.0)
        nc.gpsimd.affine_select(
            out=ident, in_=ident, compare_op=mybir.AluOpType.is_equal,
            fill=0.0, base=0, channel_multiplier=1, pattern=[[-1, 128]],
        )

        # ---------------- phases A+B (xnT + projections) ----------------
        with ExitStack() as ab:
            xnt_pool = ab.enter_context(
                tc.tile_pool(name="xnt", bufs=NT, side="right")
            )
            xsq_pool = ab.enter_context(
                tc.tile_pool(name="xsq", bufs=3, side="right")
            )
            w_pool = ab.enter_context(
                tc.tile_pool(name="wstream", bufs=NT + 4, side="right")
            )

            # single shared PSUM pool (8 banks): lets the scheduler interleave
            # the ss accumulation with early projection matmuls instead of
            # serializing phase A before phase B
            pp = ctx.enter_context(tc.tile_pool(name="pp", bufs=8, space="PSUM"))

            xnt = []
            ss = [pp.tile([128, 512], F32, tag="pp", name="ss") for _ in range(NSC)]
            for ht in range(NT):
                t = xnt_pool.tile([128, S], BF, tag="xnt", name="xnt")
                # split-row DMAs pipeline the squares behind the loads; the
                # first tile lands in quarters (parallel queues) so the first
                # matmul issues sooner
                nparts = 4 if ht == 0 else 2
                step = S // nparts
                for p_ in range(nparts):
                    nc.sync.dma_start(
                        out=t[:, p_ * step:(p_ + 1) * step],
                        in_=xt[ht * 128:(ht + 1) * 128, p_ * step:(p_ + 1) * step],
                    )
                xnt.append(t)
                sq = xsq_pool.tile([128, S], BF, tag="xsq", name="xsq")
                nc.vector.tensor_mul(sq[:, 0:S // 2], t[:, 0:S // 2], t[:, 0:S // 2])
                nc.vector.tensor_mul(sq[:, S // 2:], t[:, S // 2:], t[:, S // 2:])
                # M=128 all-ones stationary: every partition gets the column
                # sum, so rstd lands pre-broadcast
                for sc in range(NSC):
                    nc.tensor.matmul(
                        ss[sc],
                        ones_mat_bf,
                        sq[:, sc * 512:(sc + 1) * 512],
                        start=(ht == 0),
                        stop=(ht == NT - 1),
                    )
            for sc in range(NSC):
                cs = slice(sc * 512, (sc + 1) * 512)
                # sqrt(mean + eps), then reciprocal -> rstd (all lanes)
                mtmp = denb_pool.tile([128, 512], F32, tag="denb", name="mtmp")
                nc.scalar.activation(
                    mtmp, ss[sc], AF.Sqrt, bias=eps_sb, scale=1.0 / HID
                )
                nc.vector.reciprocal_approx_fast(rstd_b[:, cs], mtmp)

            # PE-transpose rstd_b slices to get per-partition rstd columns
            for st in range(NT):
                ptr = pp.tile([128, 512], F32, tag="pp", name="pp")
                nc.tensor.transpose(
                    ptr[:, 0:128], rstd_b[:, st * 128:(st + 1) * 128], ident
                )
                nc.vector.tensor_copy(rstd_colT[:, st:st + 1], ptr[:, 0:1])

            # --- q/k projections: qT/kT [dh, S] per head, stationary = W tile
            qts, kts = [], []
            for (w_dram, dst_list) in ((wq, qts), (wk, kts)):
                wts = []
                for ht in range(NT):
                    wt = w_pool.tile([128, KSH], BF, tag="w", name="w")
                    nc.sync.dma_start(
                        out=wt, in_=w_dram[ht * 128:(ht + 1) * 128, :]
                    )
                    wts.append(wt)
                for dt in range(NHS):
                    dst = (qt_pool if dst_list is qts else kt_pool).tile(
                        [128, S], BF, tag="qt", name="qt" if dst_list is qts else "kt"
                    )
                    ps = [pp.tile([128, 512], F32, tag="pp", name="pp") for _ in range(NSC)]
                    for ht in range(NT):
                        lhsT = wts[ht][:, dt * 128:(dt + 1) * 128]
                        for sc in range(NSC):
                            nc.tensor.matmul(
                                ps[sc],
                                lhsT,
                                xnt[ht][:, sc * 512:(sc + 1) * 512],
                                start=(ht == 0),
                                stop=(ht == NT - 1),
                            )
                    for sc in range(NSC):
                        cs = slice(sc * 512, (sc + 1) * 512)
                        # fold rstd[s] (free axis here) into the evacuation
                        nc.vector.tensor_mul(dst[:, cs], ps[sc], rstd_b[:, cs])
                    dst_list.append(dst)

            # --- v projection: natural layout [S, 512], stationary = xnT slice
            wvts = []
            for ht in range(NT):
                wt = w_pool.tile([128, KSH], BF, tag="w", name="w")
                nc.sync.dma_start(out=wt, in_=wv[ht * 128:(ht + 1) * 128, :])
                wvts.append(wt)
            v_sb = []
            for st in range(NT):
                psv = pp.tile([128, 512], F32, tag="pp", name="pp")
                for ht in range(NT):
                    nc.tensor.matmul(
                        psv,
                        xnt[ht][:, st * 128:(st + 1) * 128],
                        wvts[ht],
                        start=(ht == 0),
                        stop=(ht == NT - 1),
                    )
                vt = v_pool.tile([128, KSH], BF, tag="v", name="v")
                # fold rstd[s] (partition axis here) into the evacuation
                nc.vector.tensor_scalar_mul(vt, psv, rstd_colT[:, st:st + 1])
                v_sb.append(vt)
        # xnt/xsq/wstream released here

        wo_pool = ctx.enter_context(tc.tile_pool(name="wo", bufs=NHS, side="right"))
        out_pool = ctx.enter_context(tc.tile_pool(name="outp", bufs=4, side="right"))
        wo_sb = []
        for c in range(NHS):
            wt = wo_pool.tile([128, HID], BF, tag="wo", name="wo")
            nc.sync.dma_start(out=wt, in_=wo[c * 128:(c + 1) * 128, :])
            wo_sb.append(wt)

        # -------- phases C+D: attention, with o_proj interleaved per chunk ---
        # sc-outer / head-inner: once all 4 heads finish query-chunk sc, the
        # o_proj for that chunk's four s-tiles runs immediately, so the final
        # matmul phase and output DMAs overlap the remaining attention work.
        # Denominators: M=128 all-ones stationary -> every PSUM partition gets
        # the column sum (same N-cycle cost as M=1, and the result is already
        # broadcast, so the reciprocal runs full-lane straight off PSUM).
        attn_sb = [
            attn_pool.tile([128, S], BF, tag="attn", name="attn")
            for _ in range(NHS)
        ]
        for sc in range(NSC):
            cs = slice(sc * 512, (sc + 1) * 512)
            ntt = 4 * (sc + 1)
            for hd in range(NHS):
                at = attn_sb[hd]
                hs = slice(hd * 128, (hd + 1) * 128)
                ps_at = pp.tile([128, 512], F32, tag="pp", name="at")
                ps_dn = pp.tile([128, 512], F32, tag="pp", name="dn")
                for tt in range(ntt):
                    # diagonal tiles: columns below 128*j are fully masked --
                    # skip them in the matmuls / exp / mask (causal truncation)
                    j = tt - 4 * sc
                    c0 = 128 * j if (j > 0 and TRUNC) else 0
                    ps_s = pp.tile([128, 512], F32, tag="pp", name="pp")
                    nc.tensor.matmul(
                        ps_s[:, c0:],
                        kts[hd][:, tt * 128:(tt + 1) * 128],
                        qts[hd][:, sc * 512 + c0:(sc + 1) * 512],
                        start=True,
                        stop=True,
                    )
                    pt = probs_pool.tile([128, 512], BF, tag="probs", name="probs")
                    nc.scalar.activation(pt[:, c0:], ps_s[:, c0:], AF.Exp)
                    if j >= 0:
                        # keep where (f + c0) - t - 128*j >= 0 within the window
                        nc.gpsimd.affine_select(
                            out=pt[:, c0:],
                            in_=pt[:, c0:],
                            compare_op=mybir.AluOpType.is_ge,
                            fill=0.0,
                            base=c0 - 128 * j,
                            channel_multiplier=-1,
                            pattern=[[1, 512 - c0]],
                        )
                    nc.tensor.matmul(
                        ps_at[:, c0:],
                        v_sb[tt][:, hs],
                        pt[:, c0:],
                        start=(tt == 0),
                        stop=(tt == ntt - 1),
                    )
                    nc.tensor.matmul(
                        ps_dn[:, c0:],
                        ones_mat_bf,
                        pt[:, c0:],
                        start=(tt == 0),
                        stop=(tt == ntt - 1),
                    )
                denb = denb_pool.tile([128, 512], F32, tag="denb", name="denb")
                nc.vector.reciprocal_approx_fast(denb, ps_dn)
                nc.vector.tensor_mul(at[:, cs], ps_at, denb)

            # o_proj for the four s-tiles of this chunk (all heads now done)
            for st in range(4 * sc, 4 * sc + 4):
                ot = out_pool.tile([128, HID], F32, tag="outp", name="outp")
                ps_o = [
                    pp.tile([128, 512], F32, tag="pp", name="po")
                    for _ in range(NSC)
                ]
                for c in range(NHS):
                    lhsT = attn_sb[c][:, st * 128:(st + 1) * 128]
                    for ec in range(NSC):
                        nc.tensor.matmul(
                            ps_o[ec],
                            lhsT,
                            wo_sb[c][:, ec * 512:(ec + 1) * 512],
                            start=(c == 0),
                            stop=(c == NHS - 1),
                        )
                for ec in range(NSC):
                    es = slice(ec * 512, (ec + 1) * 512)
                    nc.vector.tensor_copy(ot[:, es], ps_o[ec])
                    nc.sync.dma_start(
                        out=out[st * 128:(st + 1) * 128, es], in_=ot[:, es]
                    )

    return nc


def get_nc():
    if "nc" not in _STATE:
        nc = _build_nc()
        nc.finalize()
        _STATE["nc"] = nc
    return _STATE["nc"]


def make_in_maps(x, rms_w, Wq, Wk, Wv, Wo):
    """Host-side sharding: returns one input dict per core (8 cores)."""
    bf16 = ml_dtypes.bfloat16
    scale = 1.0 / np.sqrt(np.float32(HID))
    rw = rms_w.astype(np.float32)[:, None]
    wq_f = (rw * Wq.astype(np.float32) * scale)
    wk_f = (rw * Wk.astype(np.float32))
    wv_f = (rw * Wv.astype(np.float32))
    in_maps = []
    for c in range(DP * TP):
        b, i = divmod(c, TP)
        cols = slice(i * KSH, (i + 1) * KSH)
        in_maps.append({
            "xt": np.ascontiguousarray(x[b].T).astype(bf16),
            "wq": np.ascontiguousarray(wq_f[:, cols]).astype(bf16),
            "wk": np.ascontiguousarray(wk_f[:, cols]).astype(bf16),
            "wv": np.ascontiguousarray(wv_f[:, cols]).astype(bf16),
            "wo": np.ascontiguousarray(Wo.astype(np.float32)[cols, :]).astype(bf16),
        })
    return in_maps


def kernel(x, rms_w, Wq, Wk, Wv, Wo, _trace=False, _results_out=None):
    from concourse.bass_utils import run_bass_kernel_spmd

    nc = get_nc()
    in_maps = make_in_maps(x, rms_w, Wq, Wk, Wv, Wo)
    kw = {}
    if _trace:
        kw = dict(trace=True, trace_cores=list(range(DP * TP)))
    res = run_bass_kernel_spmd(
        nc, in_maps, core_ids=list(range(DP * TP)), **kw
    )
    if _results_out is not None:
        _results_out.append(res)
    out = np.empty((DP, S, HID), np.float32)
    for b in range(DP):
        acc = x[b].astype(np.float32).copy()
        for i in range(TP):
            acc += res.results[b * TP + i]["out"]
        out[b] = acc
    return out



# revision 3
# speedup vs baseline: 1.4020x; 1.4020x over previous
"""Trainium2 Bass kernel for a single causal-attention transformer block.

fp8(e4m3) + DoubleRow rewrite of the bf16 baseline.  Reference computation
per batch element b:
    xn  = rms_norm(x[b]) * rms_w
    q/k/v = xn @ Wq/Wk/Wv            (16 heads x 128 head dim)
    att = causal_softmax(q k^T / sqrt(2048)) @ v
    out[b] = att @ Wo + x[b]

Sharding (8 NeuronCores): tensor-parallel over heads x data-parallel over
batch.  Core c handles batch b = c // 4 and head-group i = c % 4 (4 heads,
512 columns of Wq/Wk/Wv, 512 rows of Wo).  Each core computes a partial
output  att_i @ Wo_i * 64  for its batch element; the host sums the 4
partials per batch (/64) and adds the residual.

Key speed idea: all big matmuls use fp8e4 operands with
MatmulPerfMode.DoubleRow — the PE contracts TWO 128-deep k-tiles per
instruction (stationary [128,2,M], moving [128,2,N]).  Contraction dims
are pre-paired in SBUF layouts:
  - x^T and Wq/Wk/Wv/Wo are stored as [128, 2, *] "pair tiles" (host packs
    hidden/key-dim rows 256p..256p+255 into one 128-partition tile).
  - probs tiles hold two adjacent key-tiles [t,2,s]; v holds two adjacent
    key-tiles [t,2,dh*4]; attn^T holds two heads [dh,2,s].
Scores (contract dim = head dim 128) cannot pair, so q/k stay bf16 there.
Precision: weights are pre-scaled on host by powers of 2 into e4m3 range
(Wq/Wk x sqrt(scale)*512, Wv/Wo x 64); inverse scales fold into the rstd
evacuation multipliers and the host-side gather (/64).

PSUM (8 banks): pool pp2 = 3x [128,1024] (2 banks each; score pairs get
one exp per two tiles, pv+den share one tile's two banks), pool pp =
2x [128,512] (o_proj runs each s-tile in two 2-chunk waves).
"""

import numpy as np
import ml_dtypes

S = 2048          # sequence length
HID = 2048        # hidden dim
KSH = 512         # per-core key-dim shard
DH = 128          # head dim
NHS = 4           # heads per core
TP = 4            # head-group shards
DP = 2            # batch shards
NP = 8            # hidden-dim pair tiles (256 rows each)
NT = S // 128     # 16
NSC = S // 512    # 4
EPS = 1e-5
SQK = 512.0       # host pre-scale on Wq,Wk (each also carries HID**-0.25)
SVO = 64.0        # host pre-scale on Wv and Wo

_STATE = {}


def _build_nc():
    from contextlib import ExitStack

    import concourse.bacc as bacc
    import concourse.tile as tile
    from concourse import mybir

    F32 = mybir.dt.float32
    BF = mybir.dt.bfloat16
    FP8 = mybir.dt.float8e4
    AF = mybir.ActivationFunctionType
    DR = mybir.MatmulPerfMode.DoubleRow

    nc = bacc.Bacc("TRN2")
    xp_d = nc.dram_tensor("xp", [NP * 128, 2 * S], FP8, kind="ExternalInput")
    wq_d = nc.dram_tensor("wq", [NP * 128, 2 * KSH], FP8, kind="ExternalInput")
    wk_d = nc.dram_tensor("wk", [NP * 128, 2 * KSH], FP8, kind="ExternalInput")
    wv_d = nc.dram_tensor("wv", [NP * 128, 2 * KSH], FP8, kind="ExternalInput")
    wo_d = nc.dram_tensor("wo", [2 * 128, 2 * HID], FP8, kind="ExternalInput")
    out = nc.dram_tensor("out", [S, HID], BF, kind="ExternalOutput")

    with tile.TileContext(nc) as tc, ExitStack() as ctx:
        misc = ctx.enter_context(tc.tile_pool(name="misc", bufs=1))
        qt_pool = ctx.enter_context(tc.tile_pool(name="qt", bufs=NHS))
        kt_pool = ctx.enter_context(tc.tile_pool(name="kt", bufs=NHS))
        v_pool = ctx.enter_context(tc.tile_pool(name="v", bufs=NT // 2))
        at_pool = ctx.enter_context(tc.tile_pool(name="attn", bufs=2))
        pt_pool = ctx.enter_context(tc.tile_pool(name="probs", bufs=10))
        denb_pool = ctx.enter_context(tc.tile_pool(name="denb", bufs=4))

        ones8 = misc.tile([128, 2, 128], FP8, tag="ones8", name="ones8")
        nc.vector.memset(ones8, 1.0)
        eps_sb = misc.tile([128, 1], F32, tag="eps_sb", name="eps_sb")
        nc.vector.memset(eps_sb, EPS * SQK * SQK)
        # rstd_b[p, s] = rstd[s]/SQK on every partition p (free-axis layout)
        rstd_b = misc.tile([128, S], F32, tag="rstd_b", name="rstd_b")
        # rstd_colT[p, st] = rstd[st*128+p]/SVO (partition-axis layout)
        rstd_colT = misc.tile([128, NT], F32, tag="rstd_colT", name="rstd_colT")
        ident = misc.tile([128, 128], F32, tag="ident", name="ident")
        nc.vector.memset(ident, 1.0)
        nc.gpsimd.affine_select(
            out=ident, in_=ident, compare_op=mybir.AluOpType.is_equal,
            fill=0.0, base=0, channel_multiplier=1, pattern=[[-1, 128]],
        )

        # PSUM: 3x [128,1024] (6 banks) + 2x [128,512] (2 banks)
        pp2 = ctx.enter_context(tc.tile_pool(name="pp2", bufs=3, space="PSUM"))
        pp = ctx.enter_context(tc.tile_pool(name="pp", bufs=2, space="PSUM"))

        # ---------------- phases A+B (x^T pairs + rstd + projections) -------
        with ExitStack() as ab:
            xp_pool = ab.enter_context(
                tc.tile_pool(name="xp", bufs=NP, side="right")
            )
            sq_pool = ab.enter_context(
                tc.tile_pool(name="xsq", bufs=3, side="right")
            )
            w_pool = ab.enter_context(
                tc.tile_pool(name="wstream", bufs=2 * NP + 4, side="right")
            )

            xp = []
            ss = [pp2.tile([128, 1024], F32, tag="pp2", name="ss")
                  for _ in range(2)]
            for p in range(NP):
                t = xp_pool.tile([128, 2, S], FP8, tag="xp", name="xp")
                # split DMAs per pair tile pipeline the squares behind loads
                nsplit = 2 if p == 0 else 1
                hstep = S // nsplit
                for i in range(2):
                    for h in range(nsplit):
                        nc.sync.dma_start(
                            out=t[:, i, h * hstep:(h + 1) * hstep],
                            in_=xp_d[
                                p * 128:(p + 1) * 128,
                                i * S + h * hstep:i * S + (h + 1) * hstep,
                            ],
                        )
                xp.append(t)
                sq = sq_pool.tile([128, 2, S], FP8, tag="xsq", name="xsq")
                for i in range(2):
                    # split squares across DVE and ACT (both idle in phase A)
                    if (2 * p + i) % 2 == 0:
                        nc.vector.tensor_mul(sq[:, i, :], t[:, i, :], t[:, i, :])
                    else:
                        nc.scalar.activation(sq[:, i, :], t[:, i, :], AF.Square)
                # DoubleRow all-ones stationary: every PSUM partition gets the
                # column sum over BOTH halves of the pair; each 512-chunk is a
                # separate accumulation stream in its own PSUM bank
                for c in range(NSC):
                    nc.tensor.matmul(
                        ss[c // 2][:, (c % 2) * 512:(c % 2 + 1) * 512],
                        ones8,
                        sq[:, :, c * 512:(c + 1) * 512],
                        start=(p == 0),
                        stop=(p == NP - 1),
                        perf_mode=DR,
                    )
            for c in range(NSC):
                cs = slice(c * 512, (c + 1) * 512)
                # SQK*sqrt(mean+eps), then reciprocal -> rstd/SQK (all lanes)
                mtmp = denb_pool.tile([128, 512], F32, tag="denb", name="mtmp")
                nc.scalar.activation(
                    mtmp, ss[c // 2][:, (c % 2) * 512:(c % 2 + 1) * 512],
                    AF.Sqrt, bias=eps_sb, scale=SQK * SQK / HID,
                )
                nc.vector.reciprocal_approx_fast(rstd_b[:, cs], mtmp)

            # PE-transpose rstd_b slices to per-partition layout (x SQK/SVO)
            for st in range(NT):
                ptr = pp.tile([128, 512], F32, tag="pp", name="ptr")
                nc.tensor.transpose(
                    ptr[:, 0:128], rstd_b[:, st * 128:(st + 1) * 128], ident
                )
                nc.vector.tensor_scalar_mul(
                    rstd_colT[:, st:st + 1], ptr[:, 0:1], SQK / SVO
                )

            # --- q/k projections: qT/kT [dh, S] per head (bf16), DR matmuls
            qts, kts = [], []
            for (w_dram, dst_list) in ((wq_d, qts), (wk_d, kts)):
                wts = []
                for p in range(NP):
                    wt = w_pool.tile([128, 2, KSH], FP8, tag="w", name="w")
                    nc.sync.dma_start(
                        out=wt[:, 0, :], in_=w_dram[p * 128:(p + 1) * 128, 0:KSH]
                    )
                    nc.sync.dma_start(
                        out=wt[:, 1, :],
                        in_=w_dram[p * 128:(p + 1) * 128, KSH:2 * KSH],
                    )
                    wts.append(wt)
                for dt in range(NHS):
                    dst = (qt_pool if dst_list is qts else kt_pool).tile(
                        [128, S], BF, tag="qt",
                        name="qt" if dst_list is qts else "kt",
                    )
                    ps = [pp2.tile([128, 1024], F32, tag="pp2", name="psqk")
                          for _ in range(2)]
                    for p in range(NP):
                        lhsT = wts[p][:, :, dt * 128:(dt + 1) * 128]
                        for c in range(NSC):
                            nc.tensor.matmul(
                                ps[c // 2][:, (c % 2) * 512:(c % 2 + 1) * 512],
                                lhsT,
                                xp[p][:, :, c * 512:(c + 1) * 512],
                                start=(p == 0),
                                stop=(p == NP - 1),
                                perf_mode=DR,
                            )
                    for h in range(2):
                        cs = slice(h * 1024, (h + 1) * 1024)
                        # fold rstd/SQK (free axis here) into the evacuation
                        nc.vector.tensor_mul(dst[:, cs], ps[h], rstd_b[:, cs])
                    dst_list.append(dst)

            # --- v projection: pair tiles [t,2,KSH] fp8, stationary = xp slice
            wvts = []
            for p in range(NP):
                wt = w_pool.tile([128, 2, KSH], FP8, tag="w", name="wv")
                nc.sync.dma_start(
                    out=wt[:, 0, :], in_=wv_d[p * 128:(p + 1) * 128, 0:KSH]
                )
                nc.sync.dma_start(
                    out=wt[:, 1, :], in_=wv_d[p * 128:(p + 1) * 128, KSH:2 * KSH]
                )
                wvts.append(wt)
            v_sb = [
                v_pool.tile([128, 2, KSH], FP8, tag="v", name="v")
                for _ in range(NT // 2)
            ]
            for st in range(NT):
                psv = pp.tile([128, 512], F32, tag="pp", name="psv")
                for p in range(NP):
                    nc.tensor.matmul(
                        psv,
                        xp[p][:, :, st * 128:(st + 1) * 128],
                        wvts[p],
                        start=(p == 0),
                        stop=(p == NP - 1),
                        perf_mode=DR,
                    )
                # fold rstd/SVO (partition axis here) into the evacuation
                nc.vector.tensor_scalar_mul(
                    v_sb[st // 2][:, st % 2, :], psv, rstd_colT[:, st:st + 1]
                )
        # xp/xsq/wstream released here

        wo_pool = ctx.enter_context(tc.tile_pool(name="wo", bufs=2, side="right"))
        out_pool = ctx.enter_context(tc.tile_pool(name="outp", bufs=4, side="right"))
        wo_sb = []
        for hp in range(2):
            wt = wo_pool.tile([128, 2, HID], FP8, tag="wo", name="wo")
            nc.sync.dma_start(
                out=wt[:, 0, :], in_=wo_d[hp * 128:(hp + 1) * 128, 0:HID]
            )
            nc.sync.dma_start(
                out=wt[:, 1, :], in_=wo_d[hp * 128:(hp + 1) * 128, HID:2 * HID]
            )
            wo_sb.append(wt)

        # -------- phases C+D: attention + interleaved o_proj ---------------
        # attn^T head-pair tiles [dh, 2, S] fp8 (o_proj DR stationary)
        at_pair = [
            at_pool.tile([128, 2, S], FP8, tag="attn", name="attn")
            for _ in range(2)
        ]
        for sc in range(NSC):
            swin = slice(sc * 512, (sc + 1) * 512)
            ntt = 4 * (sc + 1)
            for hd in range(NHS):
                # pv-accum and den-accum share one [128,1024] tile: two
                # independent accumulation streams in its two banks
                atdn = pp2.tile([128, 1024], F32, tag="pp2", name="atdn")
                ps_at = atdn[:, 0:512]
                ps_dn = atdn[:, 512:1024]
                for ttp in range(ntt // 2):
                    ptp = pt_pool.tile([128, 2, 512], FP8, tag="probs", name="probs")
                    ps2 = pp2.tile([128, 1024], F32, tag="pp2", name="ps2")
                    diag = 2 * ttp + 1 - 4 * sc >= 0
                    for i in range(2):
                        tt = 2 * ttp + i
                        j = tt - 4 * sc
                        c0 = 128 * j if j > 0 else 0
                        nc.tensor.matmul(
                            ps2[:, i * 512 + c0:(i + 1) * 512],
                            kts[hd][:, tt * 128:(tt + 1) * 128],
                            qts[hd][:, sc * 512 + c0:(sc + 1) * 512],
                            start=True,
                            stop=True,
                        )
                        if diag:
                            nc.scalar.activation(
                                ptp[:, i, c0:], ps2[:, i * 512 + c0:(i + 1) * 512],
                                AF.Exp,
                            )
                            if c0 > 0:
                                # DR moving operand reads the whole pair tile:
                                # zero the fully-masked region exp never wrote
                                nc.gpsimd.memset(ptp[:, i, 0:c0], 0.0)
                            if j >= 0:
                                # keep where (f + c0) - t - 128*j >= 0
                                nc.gpsimd.affine_select(
                                    out=ptp[:, i, c0:],
                                    in_=ptp[:, i, c0:],
                                    compare_op=mybir.AluOpType.is_ge,
                                    fill=0.0,
                                    base=c0 - 128 * j,
                                    channel_multiplier=-1,
                                    pattern=[[1, 512 - c0]],
                                )
                    if not diag:
                        # off-diagonal pair: one exp over both PSUM banks
                        nc.scalar.activation(ptp[:, :, :], ps2[:, :], AF.Exp)
                    nc.tensor.matmul(
                        ps_at,
                        v_sb[ttp][:, :, hd * 128:(hd + 1) * 128],
                        ptp[:, :, :],
                        start=(ttp == 0),
                        stop=(ttp == ntt // 2 - 1),
                        perf_mode=DR,
                    )
                    nc.tensor.matmul(
                        ps_dn,
                        ones8,
                        ptp[:, :, :],
                        start=(ttp == 0),
                        stop=(ttp == ntt // 2 - 1),
                        perf_mode=DR,
                    )
                denb = denb_pool.tile([128, 512], F32, tag="denb", name="denb")
                nc.vector.reciprocal_approx_fast(denb, ps_dn)
                nc.vector.tensor_mul(
                    at_pair[hd // 2][:, hd % 2, swin], ps_at, denb
                )

            # o_proj for the four s-tiles of this chunk (all heads now done);
            # two 2-chunk waves per s-tile so each wave fits the pp pool
            for st in range(4 * sc, 4 * sc + 4):
                ot = out_pool.tile([128, HID], BF, tag="outp", name="outp")
                for wave in range(2):
                    pw = [pp.tile([128, 512], F32, tag="pp", name="po")
                          for _ in range(2)]
                    for hp in range(2):
                        lhsT = at_pair[hp][:, :, st * 128:(st + 1) * 128]
                        for k in range(2):
                            ec = 2 * wave + k
                            nc.tensor.matmul(
                                pw[k],
                                lhsT,
                                wo_sb[hp][:, :, ec * 512:(ec + 1) * 512],
                                start=(hp == 0),
                                stop=(hp == 1),
                                perf_mode=DR,
                            )
                    for k in range(2):
                        ec = 2 * wave + k
                        es = slice(ec * 512, (ec + 1) * 512)
                        nc.vector.tensor_copy(ot[:, es], pw[k])
                        nc.sync.dma_start(
                            out=out[st * 128:(st + 1) * 128, es], in_=ot[:, es]
                        )

    return nc


def get_nc():
    if "nc" not in _STATE:
        nc = _build_nc()
        nc.finalize()
        _STATE["nc"] = nc
    return _STATE["nc"]


def _pack_pairs(a, blk):
    """[2*NP_blk*128, C] -> [NP_blk*128, 2*C]: rows 256p+128i+r -> [p*128+r, i*C+c]."""
    n2, c = a.shape
    npairs = n2 // 256
    a = a.reshape(npairs, 2, 128, c)          # [p, i, r, c]
    a = a.transpose(0, 2, 1, 3)               # [p, r, i, c]
    return np.ascontiguousarray(a.reshape(npairs * 128, 2 * c))


def make_in_maps(x, rms_w, Wq, Wk, Wv, Wo):
    """Host-side sharding: returns one input dict per core (8 cores)."""
    fp8 = ml_dtypes.float8_e4m3fn
    sqk = np.float32(float(HID) ** -0.25)
    rw = rms_w.astype(np.float32)[:, None]
    wq_f = rw * Wq.astype(np.float32) * (sqk * SQK)
    wk_f = rw * Wk.astype(np.float32) * (sqk * SQK)
    wv_f = rw * Wv.astype(np.float32) * SVO
    wo_f = Wo.astype(np.float32) * SVO
    in_maps = []
    for c in range(DP * TP):
        b, i = divmod(c, TP)
        cols = slice(i * KSH, (i + 1) * KSH)
        in_maps.append({
            "xp": _pack_pairs(
                np.ascontiguousarray(x[b].astype(np.float32).T), 128
            ).astype(fp8),
            "wq": _pack_pairs(wq_f[:, cols], 128).astype(fp8),
            "wk": _pack_pairs(wk_f[:, cols], 128).astype(fp8),
            "wv": _pack_pairs(wv_f[:, cols], 128).astype(fp8),
            "wo": _pack_pairs(wo_f[cols, :], 128).astype(fp8),
        })
    return in_maps


def kernel(x, rms_w, Wq, Wk, Wv, Wo, _trace=False, _results_out=None):
    from concourse.bass_utils import run_bass_kernel_spmd

    nc = get_nc()
    in_maps = make_in_maps(x, rms_w, Wq, Wk, Wv, Wo)
    kw = {}
    if _trace:
        kw = dict(trace=True, trace_cores=list(range(DP * TP)))
    res = run_bass_kernel_spmd(
        nc, in_maps, core_ids=list(range(DP * TP)), **kw
    )
    if _results_out is not None:
        _results_out.append(res)
    inv = np.float32(1.0 / SVO)
    out = np.empty((DP, S, HID), np.float32)
    for b in range(DP):
        acc = x[b].astype(np.float32).copy()
        for i in range(TP):
            acc += res.results[b * TP + i]["out"].astype(np.float32) * inv
        out[b] = acc
    return out
